# revision 1
# baseline (speedup 1.0000x reference)
"""Trainium2 Bass kernel for nn_DecoderLayer (Performer/FAVOR+ decoder layer).

Sharding: data-parallel over sequence (L) across 8 cores. The FAVOR+ attention
global statistics (kv = sum_l kp (x) v and ksum = sum_l kp, per batch element)
are the only cross-core quantities; they are AllReduced (~2MB) while the
query-side projections compute. Everything else (projections, LayerNorms, FFN)
is token-local.

Device layout: all activations live feature-major [D(partitions x chunks),
tokens(free)]; the host pre-transposes input shards and post-transposes the
output, so the device never transposes anything. Matmuls run as float32r
(full fp32 storage, ~1 cycle/row PE rate).
"""
import sys
import os

sys.path.insert(0, '/opt/trn_rl_repo')

import numpy as np
import ml_dtypes
from contextlib import ExitStack

from concourse import bass, bacc, tile
import concourse.mybir as mybir
from concourse.bass_utils import run_bass_kernel_spmd
from concourse.alu_op_type import AluOpType

F32 = mybir.dt.float32
F32R = mybir.dt.float32r
AF = mybir.ActivationFunctionType
BF16 = mybir.dt.bfloat16

B, L, D, H, DH, M, DFF = 4, 4096, 1024, 16, 64, 8, 4096
NCORES = 8
LSH = L // NCORES          # 512 tokens of L per core
T = B * LSH                # 2048 tokens per core
TB = LSH                   # token block = one batch element's shard (512)
DC = D // 128              # 8 d-chunks
HM = H * M                 # 128
EPS_LN = 1e-6
STAB = 0.001
NEWTON = True              # Newton-refine Rsqrt/Reciprocal LUT outputs

_cache = {}


def _mm(nc, out, lhsT, rhs, start, stop, skip_group_check=False):
    nc.tensor.matmul(out, lhsT.bitcast(F32R), rhs.bitcast(F32R),
                     start=start, stop=stop, skip_group_check=skip_group_check)


def build_program():
    nc = bacc.Bacc("TRN2", target_bir_lowering=False, debug=False,
                   num_devices=NCORES)

    def din(name, shape, dt=F32R):
        return nc.dram_tensor(name, shape, dt, kind="ExternalInput").ap()

    xT = din("xT", [D, T])
    encT = din("encT", [D, T])
    wq1 = din("wq1", [D, D]); wk1 = din("wk1", [D, D]); wv1 = din("wv1", [D, D])
    wo1 = din("wo1", [D, D]); bd1 = din("bd1", [D, HM])
    wq2 = din("wq2", [D, D]); wk2 = din("wk2", [D, D]); wv2 = din("wv2", [D, D])
    wo2 = din("wo2", [D, D]); bd2 = din("bd2", [D, HM])
    e16T_d = din("e16T", [HM, H]); e16_d = din("e16", [H, HM])
    kvmask_d = din("kvmask", [HM, D], F32)
    w1 = din("w1", [D, DFF]); w2 = din("w2", [DFF, D], BF16)
    b1r_d = din("b1r", [1, DFF]); b2r_d = din("b2r", [1, D])
    gbe_d = din("gbe", [128, 6 * DC], F32)  # g1|be1|g2|be2|g3|be3, chunk-packed
    ones_col_d = din("ones_col", [128, 8])
    ones_row_d = din("ones_row", [1, 128])
    ones_tb_d = din("ones_tb", [1, TB])

    outT = nc.dram_tensor("outT", [D, T], F32, kind="ExternalOutput").ap()

    with nc.allow_low_precision(reason="f32r matmul inputs (4-byte storage)"), \
         tile.TileContext(nc) as tc, ExitStack() as top:
        dram = top.enter_context(tc.tile_pool(name="dram", bufs=1, space="DRAM"))
        h_spill = dram.tile([DFF, T], BF16)
        out2_spill = dram.tile([D, T], F32R)
        arin1 = dram.tile([HM, B * (D + 1)], F32)
        arout1 = dram.tile([HM, B * (D + 1)], F32, addr_space="Shared")
        arin2 = dram.tile([HM, B * (D + 1)], F32)
        arout2 = dram.tile([HM, B * (D + 1)], F32, addr_space="Shared")

        const = top.enter_context(tc.tile_pool(name="const", bufs=1))
        e16T = const.tile([HM, H], F32R); nc.sync.dma_start(e16T[:], e16T_d[:])
        e16 = const.tile([H, HM], F32R); nc.sync.dma_start(e16[:], e16_d[:])
        gbe = const.tile([128, 6 * DC], F32); nc.sync.dma_start(gbe[:], gbe_d[:])
        ones_col = const.tile([128, 8], F32R); nc.sync.dma_start(ones_col[:], ones_col_d[:])
        ones_row = const.tile([1, 128], F32R); nc.sync.dma_start(ones_row[:], ones_row_d[:])
        ones_tb = const.tile([1, TB], F32R); nc.sync.dma_start(ones_tb[:], ones_tb_d[:])
        eps_t = const.tile([1, 1], F32); nc.vector.memset(eps_t[:], EPS_LN)

        def gslice(i):   # per-partition [128,1] scale slice for LN i (0,1,2)
            return gbe[:, 2 * i * DC:(2 * i + 1) * DC]

        def beslice(i):
            return gbe[:, (2 * i + 1) * DC:(2 * i + 2) * DC]

        # residual stream: per-batch [128, DC*TB] tiles, feature-major
        # layout: tile[p, kc*TB + t] = act[kc*128+p, b*TB+t]
        # `mid` closes before the W2 phase so its SBUF is released.
        mid = ExitStack()
        resid = mid.enter_context(tc.tile_pool(name="resid", bufs=5))
        qp_pool = mid.enter_context(tc.tile_pool(name="qp", bufs=1))

        def load_wide(pool, src_dram, ncols, name):
            """DRAM [D, ncols] -> SBUF [128, DC*ncols], chunk kc at cols kc*ncols."""
            t_ = pool.tile([128, DC * ncols], F32R, name=name)
            for kc in range(DC):
                nc.sync.dma_start(t_[:, kc * ncols:(kc + 1) * ncols],
                                  src_dram[kc * 128:(kc + 1) * 128, :])
            return t_

        # ---------------- P0: load x feature-major, per batch ----------------
        x_b = []
        for b in range(B):
            xb = resid.tile([128, DC * TB], F32R, tag="resid", name=f"x{b}")
            for kc in range(DC):
                nc.sync.dma_start(xb[:, kc * TB:(kc + 1) * TB],
                                  xT[kc * 128:(kc + 1) * 128, b * TB:(b + 1) * TB])
            x_b.append(xb)

        def kv_phase(wk_d, wv_d, bd_d, inp_b, arin, suffix):
            """K/V projections, kp features, kv-junk accumulation, AR input.

            Two passes so Wk and Wv are not co-resident: A) K + kp for all
            batches (kp kept, 8KB/part); B) V + kv-junk accumulation."""
            with ExitStack() as ph:
                kpp = ph.enter_context(tc.tile_pool(name=f"kpp{suffix}", bufs=16))
                kp_t = {}
                with ExitStack() as pa:
                    wp = pa.enter_context(tc.tile_pool(name=f"wkp{suffix}", bufs=1))
                    work = pa.enter_context(tc.tile_pool(name=f"kvw{suffix}", bufs=1))
                    gps = pa.enter_context(tc.tile_pool(name=f"gpsk{suffix}", bufs=3, space="PSUM"))
                    kps = pa.enter_context(tc.tile_pool(name=f"kps{suffix}", bufs=2, space="PSUM"))
                    wk_sb = load_wide(wp, wk_d, D, f"wk{suffix}")
                    bd_sb = load_wide(wp, bd_d, HM, f"bd{suffix}")
                    for b in range(B):
                        xb = inp_b[b]
                        kf = work.tile([128, DC * TB], F32R, tag="kf", name=f"kf{suffix}{b}")
                        for mc in range(DC):
                            ps = gps.tile([128, TB], F32, tag="g", name=f"kps_{suffix}")
                            for kc in range(DC):
                                _mm(nc, ps[:], wk_sb[:, kc * D + mc * 128: kc * D + mc * 128 + 128],
                                    xb[:, kc * TB:(kc + 1) * TB], kc == 0, kc == DC - 1)
                            nc.any.tensor_copy(kf[:, mc * TB:(mc + 1) * TB], ps[:])
                        for ts in range(TB // 128):
                            kpt = kpp.tile([128, HM], F32R, tag="kp", name=f"kp{suffix}_{b}_{ts}")
                            psk = kps.tile([128, HM], F32, tag="kpps", name=f"kpps{suffix}")
                            for kc in range(DC):
                                _mm(nc, psk[:],
                                    kf[:, kc * TB + ts * 128: kc * TB + ts * 128 + 128],
                                    bd_sb[:, kc * HM: (kc + 1) * HM],
                                    kc == 0, kc == DC - 1)
                            nc.vector.tensor_scalar(kpt[:], psk[:], 0.0, STAB,
                                                    AluOpType.max, AluOpType.add)
                            kp_t[(b, ts)] = kpt

                with ExitStack() as pb:
                    wp = pb.enter_context(tc.tile_pool(name=f"wvp{suffix}", bufs=1))
                    vp = pb.enter_context(tc.tile_pool(name=f"vtp{suffix}", bufs=3))
                    kvo = pb.enter_context(tc.tile_pool(name=f"kvo{suffix}", bufs=2))
                    gps = pb.enter_context(tc.tile_pool(name=f"gpsv{suffix}", bufs=3, space="PSUM"))
                    kvps = pb.enter_context(tc.tile_pool(name=f"kvps{suffix}", bufs=1, space="PSUM"))
                    wv_sb = load_wide(wp, wv_d, D, f"wv{suffix}")
                    for b in range(B):
                        xb = inp_b[b]
                        kvjA = kvps.tile([128, 512], F32, tag="A", name=f"kvjA{suffix}")
                        kvjB = kvps.tile([128, 512], F32, tag="Bt", name=f"kvjB{suffix}")
                        kvjS = kvps.tile([128, 8], F32, tag="S", name=f"kvjS{suffix}")
                        for ts in range(TB // 128):
                            vt = vp.tile([128, D], F32R, tag="vt", name=f"vt{suffix}")
                            for nb in range(2):
                                ps = gps.tile([128, 512], F32, tag="g", name=f"vps_{suffix}")
                                for kc in range(DC):
                                    _mm(nc, ps[:],
                                        xb[:, kc * TB + ts * 128: kc * TB + ts * 128 + 128],
                                        wv_sb[:, kc * D + nb * 512: kc * D + nb * 512 + 512],
                                        kc == 0, kc == DC - 1)
                                nc.any.tensor_copy(vt[:, nb * 512:(nb + 1) * 512], ps[:])

                            kpt = kp_t[(b, ts)]
                            first, last = ts == 0, ts == TB // 128 - 1
                            _mm(nc, kvjA[:], kpt[:], vt[:, 0:512], first, last, True)
                            _mm(nc, kvjB[:], kpt[:], vt[:, 512:1024], first, last, True)
                            _mm(nc, kvjS[:], kpt[:], ones_col[:], first, last, True)

                        kvj = kvo.tile([128, D + 1], F32, tag="kvj", name=f"kvj{suffix}")
                        nc.any.tensor_copy(kvj[:, 0:512], kvjA[:])
                        nc.any.tensor_copy(kvj[:, 512:1024], kvjB[:])
                        nc.any.tensor_copy(kvj[:, 1024:1025], kvjS[:, 0:1])
                        nc.sync.dma_start(arin[:, b * (D + 1):(b + 1) * (D + 1)], kvj[:])

        def q_phase(wq_d, bd_d, inp_b, qp_fm, suffix, from_dram=None):
            """Q projection + qp features -> qp_fm [128, T] feature-major."""
            with ExitStack() as ph:
                wp = ph.enter_context(tc.tile_pool(name=f"wq{suffix}", bufs=1))
                work = ph.enter_context(tc.tile_pool(name=f"qw{suffix}", bufs=2))
                gps = ph.enter_context(tc.tile_pool(name=f"qgps{suffix}", bufs=2, space="PSUM"))
                qps_ = ph.enter_context(tc.tile_pool(name=f"qpps{suffix}", bufs=2, space="PSUM"))

                wq_sb = load_wide(wp, wq_d, D, f"wqw{suffix}")
                bd_sb = load_wide(wp, bd_d, HM, f"bdq{suffix}")

                for b in range(B):
                    if from_dram is not None:
                        xb = work.tile([128, DC * TB], F32R, tag="encb", name=f"encb{suffix}")
                        for kc in range(DC):
                            nc.sync.dma_start(
                                xb[:, kc * TB:(kc + 1) * TB],
                                from_dram[kc * 128:(kc + 1) * 128, b * TB:(b + 1) * TB])
                    else:
                        xb = inp_b[b]
                    qf = work.tile([128, DC * TB], F32R, tag="qf", name=f"qf{suffix}", bufs=1)
                    for mc in range(DC):
                        ps = gps.tile([128, TB], F32, tag="g", name=f"qps_{suffix}")
                        for kc in range(DC):
                            _mm(nc, ps[:], wq_sb[:, kc * D + mc * 128: kc * D + mc * 128 + 128],
                                xb[:, kc * TB:(kc + 1) * TB], kc == 0, kc == DC - 1)
                        nc.any.tensor_copy(qf[:, mc * TB:(mc + 1) * TB], ps[:])
                    pq = qps_.tile([128, TB], F32, tag="qp", name=f"qpps_{suffix}")
                    for kc in range(DC):
                        _mm(nc, pq[:], bd_sb[:, kc * HM:(kc + 1) * HM],
                            qf[:, kc * TB:(kc + 1) * TB], kc == 0, kc == DC - 1)
                    nc.vector.tensor_scalar(qp_fm[:, b * TB:(b + 1) * TB], pq[:],
                                            0.0, STAB, AluOpType.max, AluOpType.add)

        def favor_out_phase(wo_d, arout, qp_fm, inp_b, out_b_list, ln_idx, suffix,
                            spill_to=None):
            """o = (qp/z) @ kv per head, o-proj, residual + LN -> out_b tiles."""
            with ExitStack() as ph:
                wp = ph.enter_context(tc.tile_pool(name=f"wo{suffix}", bufs=1))
                kvp = ph.enter_context(tc.tile_pool(name=f"kvi{suffix}", bufs=2))
                bdkvp = ph.enter_context(tc.tile_pool(name=f"bdkv{suffix}", bufs=1))
                fv = ph.enter_context(tc.tile_pool(name=f"fv{suffix}", bufs=1))
                ofm = ph.enter_context(tc.tile_pool(name=f"ofm{suffix}", bufs=1))
                r1p = ph.enter_context(tc.tile_pool(name=f"r1{suffix}", bufs=1))
                sqp = ph.enter_context(tc.tile_pool(name=f"sq{suffix}", bufs=2))
                stp = ph.enter_context(tc.tile_pool(name=f"st{suffix}", bufs=8))
                gps = ph.enter_context(tc.tile_pool(name=f"ogps{suffix}", bufs=3, space="PSUM"))
                sps = ph.enter_context(tc.tile_pool(name=f"osps{suffix}", bufs=5, space="PSUM"))

                wo_sb = load_wide(wp, wo_d, D, f"wow{suffix}")
                kvmask = kvp.tile([HM, D], F32, tag="kvmask", name=f"kvmask{suffix}")
                nc.sync.dma_start(kvmask[:], kvmask_d[:])

                for b in range(B):
                    bs = b * (D + 1)
                    kvb = kvp.tile([HM, D + 1], F32, tag="kvb", name=f"kvb{suffix}")
                    nc.sync.dma_start(kvb[:], arout[:, bs:bs + D + 1])
                    bdkv = bdkvp.tile([HM, D], F32R, tag="bdkv", name=f"bdkv_{suffix}")
                    nc.vector.tensor_tensor(bdkv[:], kvb[:, 0:D], kvmask[:],
                                            AluOpType.mult)
                    # z = e16T^T @ (qp * ksum_col) ; per-partition scalar = ksum
                    qpk = fv.tile([128, TB], F32R, tag="qpk", name=f"qpk{suffix}")
                    nc.vector.tensor_scalar(qpk[:], qp_fm[:, b * TB:(b + 1) * TB],
                                            kvb[:, D:D + 1], None,
                                            AluOpType.mult)
                    zps = sps.tile([H, TB], F32, tag="s", name=f"z{suffix}")
                    _mm(nc, zps[:], e16T[:], qpk[:], True, True)
                    rz = fv.tile([H, TB], F32R, tag="rz", name=f"rz{suffix}")
                    nc.vector.reciprocal(rz[:], zps[:])
                    if NEWTON:
                        t1 = fv.tile([H, TB], F32, tag="nt1", name=f"nt1{suffix}")
                        nc.vector.tensor_tensor(t1[:], zps[:], rz[:], AluOpType.mult)
                        nc.vector.tensor_scalar(t1[:], t1[:], -1.0, 2.0,
                                                AluOpType.mult, AluOpType.add)
                        nc.vector.tensor_tensor(rz[:], rz[:], t1[:], AluOpType.mult)
                    zbc = sps.tile([128, TB], F32, tag="s", name=f"zbc{suffix}")
                    _mm(nc, zbc[:], e16[:], rz[:], True, True)
                    qps_t = fv.tile([128, TB], F32R, tag="qps", name=f"qps{suffix}")
                    nc.vector.tensor_tensor(qps_t[:], qp_fm[:, b * TB:(b + 1) * TB],
                                            zbc[:], AluOpType.mult)

                    # o feature-major via block-diag kv
                    of = ofm.tile([128, DC * TB], F32R, tag="of", name=f"of{suffix}")
                    for c in range(DC):
                        ps = gps.tile([128, TB], F32, tag="g", name=f"ops_{suffix}")
                        _mm(nc, ps[:], bdkv[:, c * 128:(c + 1) * 128], qps_t[:],
                            True, True)
                        nc.any.tensor_copy(of[:, c * TB:(c + 1) * TB], ps[:])

                    # o-proj + residual + LN stats
                    r1 = r1p.tile([128, DC * TB], F32R, tag="r1", name=f"r1{suffix}")
                    Sp = sps.tile([1, TB], F32, tag="s", name=f"S{suffix}")
                    SSp = sps.tile([1, TB], F32, tag="s", name=f"SS{suffix}")
                    for mc in range(DC):
                        ps = gps.tile([128, TB], F32, tag="g", name=f"ojps_{suffix}")
                        for kc in range(DC):
                            _mm(nc, ps[:], wo_sb[:, kc * D + mc * 128: kc * D + mc * 128 + 128],
                                of[:, kc * TB:(kc + 1) * TB], kc == 0, kc == DC - 1)
                        nc.vector.tensor_tensor(r1[:, mc * TB:(mc + 1) * TB], ps[:],
                                                inp_b[b][:, mc * TB:(mc + 1) * TB],
                                                AluOpType.add)
                        sq = sqp.tile([128, TB], F32R, tag="sq", name=f"sq{suffix}")
                        nc.scalar.activation(sq[:], r1[:, mc * TB:(mc + 1) * TB], AF.Square)
                        _mm(nc, Sp[:], ones_col[:, 0:1], r1[:, mc * TB:(mc + 1) * TB],
                            mc == 0, mc == DC - 1, True)
                        _mm(nc, SSp[:], ones_col[:, 0:1], sq[:], mc == 0, mc == DC - 1, True)

                    # stats -> a (rstd), bb (-m*rstd)
                    mneg = stp.tile([1, TB], F32, tag="st", name=f"mneg{suffix}")
                    nc.vector.tensor_scalar(mneg[:], Sp[:], -1.0 / D, None, AluOpType.mult)
                    m2 = stp.tile([1, TB], F32, tag="st", name=f"m2{suffix}")
                    nc.vector.tensor_tensor(m2[:], mneg[:], mneg[:], AluOpType.mult)
                    ve = stp.tile([1, TB], F32, tag="st", name=f"ve{suffix}")
                    nc.vector.scalar_tensor_tensor(ve[:], in0=SSp[:], scalar=1.0 / D,
                                                   in1=m2[:], op0=AluOpType.mult,
                                                   op1=AluOpType.subtract)
                    sqv = stp.tile([1, TB], F32, tag="st", name=f"sqv{suffix}")
                    nc.scalar.activation(sqv[:], ve[:], AF.Sqrt, bias=eps_t[:])
                    a_ = stp.tile([1, TB], F32R, tag="st", name=f"a{suffix}")
                    nc.vector.reciprocal(a_[:], sqv[:])
                    if NEWTON:
                        n1 = stp.tile([1, TB], F32, tag="st", name=f"n1{suffix}")
                        nc.vector.tensor_tensor(n1[:], a_[:], a_[:], AluOpType.mult)
                        n2 = stp.tile([1, TB], F32, tag="st", name=f"n2{suffix}")
                        nc.vector.scalar_tensor_tensor(n2[:], in0=ve[:], scalar=EPS_LN,
                                                       in1=n1[:], op0=AluOpType.add,
                                                       op1=AluOpType.mult)
                        nc.vector.tensor_scalar(n2[:], n2[:], -0.5, 1.5,
                                                AluOpType.mult, AluOpType.add)
                        nc.vector.tensor_tensor(a_[:], a_[:], n2[:], AluOpType.mult)
                    bb = stp.tile([1, TB], F32R, tag="st", name=f"bb{suffix}")
                    nc.vector.tensor_tensor(bb[:], mneg[:], a_[:], AluOpType.mult)
                    abc = sps.tile([128, TB], F32, tag="s", name=f"abc{suffix}")
                    _mm(nc, abc[:], ones_row[:], a_[:], True, True)
                    bbc = sps.tile([128, TB], F32, tag="s", name=f"bbc{suffix}")
                    _mm(nc, bbc[:], ones_row[:], bb[:], True, True)

                    ob = resid.tile([128, DC * TB], F32R, tag="resid",
                                    name=f"out{ln_idx}_{b}")
                    for mc in range(DC):
                        tpm = sqp.tile([128, TB], F32, tag="sq", name=f"tpm{suffix}")
                        nc.vector.tensor_tensor(tpm[:], r1[:, mc * TB:(mc + 1) * TB],
                                                abc[:], AluOpType.mult)
                        nc.vector.tensor_tensor(tpm[:], tpm[:], bbc[:], AluOpType.add)
                        nc.scalar.activation(ob[:, mc * TB:(mc + 1) * TB], tpm[:],
                                             AF.Identity, bias=beslice(ln_idx)[:, mc:mc + 1],
                                             scale=gslice(ln_idx)[:, mc:mc + 1])
                    if spill_to is not None:
                        for kc in range(DC):
                            nc.sync.dma_start(
                                spill_to[kc * 128:(kc + 1) * 128, b * TB:(b + 1) * TB],
                                ob[:, kc * TB:(kc + 1) * TB])
                    out_b_list.append(ob)

        def allreduce(arin, arout):
            nc.gpsimd.collective_compute(
                "AllReduce", AluOpType.add,
                replica_groups=[list(range(NCORES))],
                ins=[arin[:]], outs=[arout[:]])

        # =================== attention 1 (self) ===================
        kv_phase(wk1, wv1, bd1, x_b, arin1, "a1")
        allreduce(arin1, arout1)
        qp1 = qp_pool.tile([HM, T], F32R, tag="qp", name="qp1")
        q_phase(wq1, bd1, x_b, qp1, "a1")
        out1_b = []
        favor_out_phase(wo1, arout1, qp1, x_b, out1_b, 0, "a1")

        # =================== attention 2 (cross: q from enc, kv from out1) ===
        kv_phase(wk2, wv2, bd2, out1_b, arin2, "a2")
        allreduce(arin2, arout2)
        qp2 = qp_pool.tile([HM, T], F32R, tag="qp", name="qp2")
        q_phase(wq2, bd2, None, qp2, "a2", from_dram=encT)
        out2_b = []
        favor_out_phase(wo2, arout2, qp2, out1_b, out2_b, 1, "a2",
                        spill_to=out2_spill)

        # =================== FFN ===================
        # P7a: h = elu(out2 @ W1 + b1), spilled to DRAM feature-major [DFF, T]
        with ExitStack() as ph:
            wp = ph.enter_context(tc.tile_pool(name="w1p", bufs=2))
            hp = ph.enter_context(tc.tile_pool(name="hp", bufs=3))
            ep = ph.enter_context(tc.tile_pool(name="ep", bufs=3))
            b1p = ph.enter_context(tc.tile_pool(name="b1p", bufs=1))
            hps = ph.enter_context(tc.tile_pool(name="hps", bufs=4, space="PSUM"))
            b1row = b1p.tile([1, DFF], F32R, name="b1row")
            nc.sync.dma_start(b1row[:], b1r_d[:])
            for dffc in range(DFF // 512):
                w1c = wp.tile([128, DC * 512], F32R, tag="w1c", name="w1c")
                for kc in range(DC):
                    nc.sync.dma_start(w1c[:, kc * 512:(kc + 1) * 512],
                                      w1[kc * 128:(kc + 1) * 128,
                                         dffc * 512:(dffc + 1) * 512])
                for b in range(B):
                    for ms in range(4):
                        ps = hps.tile([128, TB], F32, tag="h", name="hps_t")
                        for kc in range(DC):
                            _mm(nc, ps[:],
                                w1c[:, kc * 512 + ms * 128: kc * 512 + ms * 128 + 128],
                                out2_b[b][:, kc * TB:(kc + 1) * TB],
                                kc == 0, False)
                        _mm(nc, ps[:],
                            b1row[0:1, dffc * 512 + ms * 128: dffc * 512 + ms * 128 + 128],
                            ones_tb[:], False, True)
                        # ELU: h = min(exp(u) - 1, max(u, 0))
                        e_ = ep.tile([128, TB], F32, tag="e", name="e_t")
                        nc.scalar.activation(e_[:], ps[:], AF.Exp)
                        t_ = ep.tile([128, TB], F32, tag="t", name="t_t")
                        nc.vector.tensor_scalar(t_[:], ps[:], 0.0, None, AluOpType.max)
                        h_ = hp.tile([128, TB], BF16, tag="hsb", name="h_t")
                        nc.vector.scalar_tensor_tensor(h_[:], in0=e_[:], scalar=1.0,
                                                       in1=t_[:], op0=AluOpType.subtract,
                                                       op1=AluOpType.min)
                        nc.sync.dma_start(
                            h_spill[dffc * 512 + ms * 128: dffc * 512 + ms * 128 + 128,
                                    b * TB:(b + 1) * TB], h_[:])

        # P7b: r3 = h @ W2 + b2 + out2 ; LN3 -> outT
        # resid/qp pools close here; W2 takes their space.
        mid.close()
        TB3 = 512
        with ExitStack() as ph:
            wp = ph.enter_context(tc.tile_pool(name="w2p", bufs=1))
            b2p = ph.enter_context(tc.tile_pool(name="b2p", bufs=1))
            hin = ph.enter_context(tc.tile_pool(name="hin", bufs=4))
            o2p = ph.enter_context(tc.tile_pool(name="o2p", bufs=2))
            r3p = ph.enter_context(tc.tile_pool(name="r3p", bufs=1))
            sqp = ph.enter_context(tc.tile_pool(name="sq3", bufs=2))
            stp = ph.enter_context(tc.tile_pool(name="st3", bufs=8))
            o3p = ph.enter_context(tc.tile_pool(name="o3p", bufs=3))
            # one shared PSUM pool: 8 r3 banks rotate with the LN3 stat tiles
            rps = ph.enter_context(tc.tile_pool(name="rps", bufs=8, space="PSUM"))

            w2_sb = wp.tile([128, (DFF // 128) * D], BF16, name="w2sb")
            for kc in range(DFF // 128):
                nc.sync.dma_start(w2_sb[:, kc * D:(kc + 1) * D],
                                  w2[kc * 128:(kc + 1) * 128, :])
            b2row = b2p.tile([1, D], F32R, name="b2row")
            nc.sync.dma_start(b2row[:], b2r_d[:])

            for t3 in range(T // TB3):
                # one psum bank per d-chunk (start=True zeroes a whole bank)
                rt = [rps.tile([128, TB3], F32, tag="r3", name=f"r3ps{i}")
                      for i in range(DC)]
                for kc in range(DFF // 128):
                    hk = hin.tile([128, TB3], BF16, tag="hk", name="hk")
                    nc.sync.dma_start(hk[:], h_spill[kc * 128:(kc + 1) * 128,
                                                     t3 * TB3:(t3 + 1) * TB3])
                    for c in range(DC):
                        nc.tensor.matmul(rt[c][:],
                            w2_sb[:, kc * D + c * 128: kc * D + c * 128 + 128],
                            hk[:], start=(kc == 0), stop=False,
                            skip_group_check=True)
                # b2 row: finish accumulation groups
                for c in range(DC):
                    _mm(nc, rt[c][:],
                        b2row[0:1, c * 128:(c + 1) * 128],
                        ones_tb[0:1, 0:TB3], False, True, True)

                r3 = r3p.tile([128, DC * TB3], F32R, tag="r3s", name="r3s")
                Sp = rps.tile([1, TB3], F32, tag="r3", name="S3")
                SSp = rps.tile([1, TB3], F32, tag="r3", name="SS3")
                for c in range(DC):
                    o2c = o2p.tile([128, TB3], F32R, tag="o2c", name="o2c")
                    nc.sync.dma_start(o2c[:], out2_spill[c * 128:(c + 1) * 128,
                                                         t3 * TB3:(t3 + 1) * TB3])
                    nc.vector.tensor_tensor(r3[:, c * TB3:(c + 1) * TB3], rt[c][:],
                                            o2c[:], AluOpType.add)
                    sq = sqp.tile([128, TB3], F32R, tag="sq3", name="sq3t")
                    nc.scalar.activation(sq[:], r3[:, c * TB3:(c + 1) * TB3], AF.Square)
                    _mm(nc, Sp[:], ones_col[:, 0:1], r3[:, c * TB3:(c + 1) * TB3],
                        c == 0, c == DC - 1, True)
                    _mm(nc, SSp[:], ones_col[:, 0:1], sq[:], c == 0, c == DC - 1, True)

                mneg = stp.tile([1, TB3], F32, tag="st3", name="mneg3")
                nc.vector.tensor_scalar(mneg[:], Sp[:], -1.0 / D, None, AluOpType.mult)
                m2 = stp.tile([1, TB3], F32, tag="st3", name="m23")
                nc.vector.tensor_tensor(m2[:], mneg[:], mneg[:], AluOpType.mult)
                ve = stp.tile([1, TB3], F32, tag="st3", name="ve3")
                nc.vector.scalar_tensor_tensor(ve[:], in0=SSp[:], scalar=1.0 / D,
                                               in1=m2[:], op0=AluOpType.mult,
                                               op1=AluOpType.subtract)
                sqv = stp.tile([1, TB3], F32, tag="st3", name="sqv3")
                nc.scalar.activation(sqv[:], ve[:], AF.Sqrt, bias=eps_t[:])
                a_ = stp.tile([1, TB3], F32R, tag="st3", name="a3")
                nc.vector.reciprocal(a_[:], sqv[:])
                if NEWTON:
                    n1 = stp.tile([1, TB3], F32, tag="st3", name="n13")
                    nc.vector.tensor_tensor(n1[:], a_[:], a_[:], AluOpType.mult)
                    n2 = stp.tile([1, TB3], F32, tag="st3", name="n23")
                    nc.vector.scalar_tensor_tensor(n2[:], in0=ve[:], scalar=EPS_LN,
                                                   in1=n1[:], op0=AluOpType.add,
                                                   op1=AluOpType.mult)
                    nc.vector.tensor_scalar(n2[:], n2[:], -0.5, 1.5,
                                            AluOpType.mult, AluOpType.add)
                    nc.vector.tensor_tensor(a_[:], a_[:], n2[:], AluOpType.mult)
                bb = stp.tile([1, TB3], F32R, tag="st3", name="bb3")
                nc.vector.tensor_tensor(bb[:], mneg[:], a_[:], AluOpType.mult)
                abc = rps.tile([128, TB3], F32, tag="r3", name="abc3")
                _mm(nc, abc[:], ones_row[:], a_[:], True, True)
                bbc = rps.tile([128, TB3], F32, tag="r3", name="bbc3")
                _mm(nc, bbc[:], ones_row[:], bb[:], True, True)

                for c in range(DC):
                    tpm = sqp.tile([128, TB3], F32, tag="sq3", name="tpm3")
                    nc.vector.tensor_tensor(tpm[:], r3[:, c * TB3:(c + 1) * TB3],
                                            abc[:], AluOpType.mult)
                    nc.vector.tensor_tensor(tpm[:], tpm[:], bbc[:], AluOpType.add)
                    o3 = o3p.tile([128, TB3], F32, tag="o3", name="o3t")
                    nc.scalar.activation(o3[:], tpm[:], AF.Identity,
                                         bias=beslice(2)[:, c:c + 1],
                                         scale=gslice(2)[:, c:c + 1])
                    nc.sync.dma_start(outT[c * 128:(c + 1) * 128,
                                           t3 * TB3:(t3 + 1) * TB3], o3[:])

    nc.compile()
    return nc


def _host_prep(inputs):
    """Build per-core in_maps from full inputs."""
    f32 = np.float32
    x = np.asarray(inputs['x'], f32)
    enc = np.asarray(inputs['enc_output'], f32)

    def bdiag(P):
        bd = np.zeros((D, HM), f32)
        pt = (np.asarray(P, f32) / np.sqrt(M)).T  # [DH, M]
        for h in range(H):
            bd[h * DH:(h + 1) * DH, h * M:(h + 1) * M] = pt
        return bd

    e16T = np.zeros((HM, H), f32)
    e16 = np.zeros((H, HM), f32)
    kvmask = np.zeros((HM, D), f32)
    for h in range(H):
        e16T[h * M:(h + 1) * M, h] = 1.0
        e16[h, h * M:(h + 1) * M] = 1.0
        kvmask[h * M:(h + 1) * M, h * DH:(h + 1) * DH] = 1.0

    gbe = np.zeros((128, 6 * DC), f32)
    for i, nm in enumerate(['g1', 'be1', 'g2', 'be2', 'g3', 'be3']):
        gbe[:, i * DC:(i + 1) * DC] = np.asarray(inputs[nm], f32).reshape(DC, 128).T

    shared = {
        'wq1': np.ascontiguousarray(np.asarray(inputs['Wq1'], f32).reshape(D, D)),
        'wk1': np.ascontiguousarray(np.asarray(inputs['Wk1'], f32).reshape(D, D)),
        'wv1': np.ascontiguousarray(np.asarray(inputs['Wv1'], f32).reshape(D, D)),
        'wo1': np.ascontiguousarray(np.asarray(inputs['Wo1'], f32).reshape(D, D)),
        'bd1': bdiag(inputs['P1']),
        'wq2': np.ascontiguousarray(np.asarray(inputs['Wq2'], f32).reshape(D, D)),
        'wk2': np.ascontiguousarray(np.asarray(inputs['Wk2'], f32).reshape(D, D)),
        'wv2': np.ascontiguousarray(np.asarray(inputs['Wv2'], f32).reshape(D, D)),
        'wo2': np.ascontiguousarray(np.asarray(inputs['Wo2'], f32).reshape(D, D)),
        'bd2': bdiag(inputs['P2']),
        'e16T': e16T, 'e16': e16, 'kvmask': kvmask,
        'w1': np.ascontiguousarray(np.asarray(inputs['W1'], f32)),
        'w2': np.ascontiguousarray(np.asarray(inputs['W2'], f32)).astype(ml_dtypes.bfloat16),
        'b1r': np.asarray(inputs['b1'], f32).reshape(1, DFF),
        'b2r': np.asarray(inputs['b2'], f32).reshape(1, D),
        'gbe': gbe,
        'ones_col': np.ones((128, 8), f32),
        'ones_row': np.ones((1, 128), f32),
        'ones_tb': np.ones((1, TB), f32),
    }

    in_maps = []
    for i in range(NCORES):
        sl = slice(i * LSH, (i + 1) * LSH)
        m = dict(shared)
        m['xT'] = np.ascontiguousarray(
            x[:, sl, :].transpose(2, 0, 1).reshape(D, T))
        m['encT'] = np.ascontiguousarray(
            enc[:, sl, :].transpose(2, 0, 1).reshape(D, T))
        in_maps.append(m)
    return in_maps


def kernel(**inputs) -> np.ndarray:
    if 'nc' not in _cache:
        _cache['nc'] = build_program()
    nc = _cache['nc']
    in_maps = _host_prep(inputs)
    res = run_bass_kernel_spmd(nc, in_maps, core_ids=list(range(NCORES)))
    out = np.empty((B, L, D), np.float32)
    for i in range(NCORES):
        o = res.results[i]['outT']  # [D, T] feature-major
        out[:, i * LSH:(i + 1) * LSH, :] = o.reshape(D, B, LSH).transpose(1, 2, 0)
    return out


if __name__ == '__main__':
    np.random.seed(0)
    print("building program...")
    build_program()
    print("OK")



# revision 2
# speedup vs baseline: 8665.1173x; 8665.1173x over previous
"""Trainium2 Bass kernel for nn_DecoderLayer (Performer/FAVOR+ decoder layer).

Sharding: data-parallel over sequence (L) across 8 cores. The FAVOR+ attention
global statistics (kv = sum_l kp (x) v and ksum = sum_l kp, per batch element)
are the only cross-core quantities; they are AllReduced per batch element
(4 x ~525KB per attention), each issued as soon as that batch's K/V pass
finishes so the collectives pipeline behind the remaining compute.

Device layout: all activations live feature-major [D(partitions x chunks),
tokens(free)]; the host pre-transposes input shards and post-transposes the
output, so the device never transposes anything. Matmuls run as float32r
(~1 cycle/row PE rate); Wo and the FFN weights are bf16 (paired with bf16
operands), halving their SBUF/DMA footprint. The FFN is fused per 512-token
block: h = elu(out2@W1+b1) stays in SBUF and is consumed by W2 immediately
(no DRAM spill of h); out2 is spilled once as bf16. The favor/LN tails and
the z-reciprocal chains are software-pipelined across batch elements so the
PE never waits on the DVE stats chains.
"""
import sys
import os

sys.path.insert(0, '/opt/trn_rl_repo')

import numpy as np
import ml_dtypes
from contextlib import ExitStack

from concourse import bass, bacc, tile
import concourse.mybir as mybir
from concourse.bass_utils import run_bass_kernel_spmd
from concourse.alu_op_type import AluOpType

F32 = mybir.dt.float32
F32R = mybir.dt.float32r
AF = mybir.ActivationFunctionType
BF16 = mybir.dt.bfloat16

B, L, D, H, DH, M, DFF = 4, 4096, 1024, 16, 64, 8, 4096
NCORES = 8
LSH = L // NCORES          # 512 tokens of L per core
T = B * LSH                # 2048 tokens per core
TB = LSH                   # token block = one batch element's shard (512)
DC = D // 128              # 8 d-chunks
HM = H * M                 # 128
EPS_LN = 1e-6
STAB = 0.001
NEWTON = True              # Newton-refine Rsqrt/Reciprocal LUT outputs

_cache = {}
VARIANT = 'full'  # full | noar | attn1 | ffn | attns


def _mm(nc, out, lhsT, rhs, start, stop, skip_group_check=False):
    nc.tensor.matmul(out, lhsT.bitcast(F32R), rhs.bitcast(F32R),
                     start=start, stop=stop, skip_group_check=skip_group_check)


def build_program():
    nc = bacc.Bacc("TRN2", target_bir_lowering=False, debug=False,
                   num_devices=NCORES)

    def din(name, shape, dt=F32R):
        return nc.dram_tensor(name, shape, dt, kind="ExternalInput").ap()

    xT = din("xT", [D, T])
    encT = din("encT", [D, T])
    wq1 = din("wq1", [D, D]); wk1 = din("wk1", [D, D]); wv1 = din("wv1", [D, D])
    wo1 = din("wo1", [D, D], BF16); bd1 = din("bd1", [D, HM])
    wq2 = din("wq2", [D, D]); wk2 = din("wk2", [D, D]); wv2 = din("wv2", [D, D])
    wo2 = din("wo2", [D, D], BF16); bd2 = din("bd2", [D, HM])
    e16T_d = din("e16T", [HM, H]); e16_d = din("e16", [H, HM])
    kvmask_d = din("kvmask", [HM, D], F32)
    w1 = din("w1", [D, DFF], BF16); w2 = din("w2", [DFF, D], BF16)
    b1r_d = din("b1r", [1, DFF], BF16); b2r_d = din("b2r", [1, D], BF16)
    gbe_d = din("gbe", [128, 6 * DC], F32)  # g1|be1|g2|be2|g3|be3, chunk-packed
    ones_col_d = din("ones_col", [128, 8])
    ones_row_d = din("ones_row", [1, 128])
    ones_tb_d = din("ones_tb", [1, TB])

    outT = nc.dram_tensor("outT", [D, T], F32, kind="ExternalOutput").ap()

    if VARIANT == 'null2':
        with tile.TileContext(nc) as tc:
            with tc.tile_pool(name="p0", bufs=2) as pool:
                t0_ = pool.tile([128, 512], F32R)
                nc.sync.dma_start(t0_[:], xT[0:128, 0:512])
                nc.sync.dma_start(outT[0:128, 0:512], t0_[:].bitcast(F32))
        nc.compile()
        return nc

    with nc.allow_low_precision(reason="f32r matmul inputs (4-byte storage)"), \
         tile.TileContext(nc) as tc, ExitStack() as top:
        dram = top.enter_context(tc.tile_pool(name="dram", bufs=1, space="DRAM"))
        out2_spill = dram.tile([D, T], BF16)
        arin1_b = [dram.tile([HM, D + 1], F32, name=f"arin1_{b}")
                   for b in range(B)]
        arout1_b = [dram.tile([HM, D + 1], F32, addr_space="Shared",
                               name=f"arout1_{b}") for b in range(B)]
        arin2_b = [dram.tile([HM, D + 1], F32, name=f"arin2_{b}")
                   for b in range(B)]
        arout2_b = [dram.tile([HM, D + 1], F32, addr_space="Shared",
                               name=f"arout2_{b}") for b in range(B)]

        const = top.enter_context(tc.tile_pool(name="const", bufs=1))
        e16T = const.tile([HM, H], F32R); nc.sync.dma_start(e16T[:], e16T_d[:])
        e16 = const.tile([H, HM], F32R); nc.sync.dma_start(e16[:], e16_d[:])
        gbe = const.tile([128, 6 * DC], F32); nc.sync.dma_start(gbe[:], gbe_d[:])
        ones_col = const.tile([128, 8], F32R); nc.sync.dma_start(ones_col[:], ones_col_d[:])
        ones_row = const.tile([1, 128], F32R); nc.sync.dma_start(ones_row[:], ones_row_d[:])
        ones_tb = const.tile([1, TB], F32R); nc.sync.dma_start(ones_tb[:], ones_tb_d[:])
        eps_t = const.tile([1, 1], F32); nc.vector.memset(eps_t[:], EPS_LN)

        def gslice(i):   # per-partition [128,1] scale slice for LN i (0,1,2)
            return gbe[:, 2 * i * DC:(2 * i + 1) * DC]

        def beslice(i):
            return gbe[:, (2 * i + 1) * DC:(2 * i + 2) * DC]

        # residual stream: per-batch [128, DC*TB] tiles, feature-major
        # layout: tile[p, kc*TB + t] = act[kc*128+p, b*TB+t]
        # `mid` closes before the W2 phase so its SBUF is released.
        mid = ExitStack()
        resid = mid.enter_context(tc.tile_pool(name="resid", bufs=5))
        qp_pool = mid.enter_context(tc.tile_pool(name="qp", bufs=1))

        def load_wide(pool, src_dram, ncols, name, dt=F32R):
            """DRAM [D, ncols] -> SBUF [128, DC*ncols], chunk kc at cols kc*ncols."""
            t_ = pool.tile([128, DC * ncols], dt, name=name)
            for kc in range(DC):
                nc.sync.dma_start(t_[:, kc * ncols:(kc + 1) * ncols],
                                  src_dram[kc * 128:(kc + 1) * 128, :])
            return t_

        # ---------------- P0: load x feature-major, per batch ----------------
        x_b = []
        for b in range(B):
            xb = resid.tile([128, DC * TB], F32R, tag="resid", name=f"x{b}")
            for kc in range(DC):
                nc.sync.dma_start(xb[:, kc * TB:(kc + 1) * TB],
                                  xT[kc * 128:(kc + 1) * 128, b * TB:(b + 1) * TB])
            x_b.append(xb)

        def kv_phase(wk_d, wv_d, bd_d, inp_b, arin_b_, suffix, arout_b=None):
            """K/V projections, kp features, kv-junk accumulation, AR input.

            Two passes so Wk and Wv are not co-resident: A) K + kp for all
            batches (kp kept, 8KB/part); B) V + kv-junk accumulation."""
            with ExitStack() as ph:
                kpp = ph.enter_context(tc.tile_pool(name=f"kpp{suffix}", bufs=16))
                kp_t = {}
                with ExitStack() as pa:
                    wp = pa.enter_context(tc.tile_pool(name=f"wkp{suffix}", bufs=1))
                    work = pa.enter_context(tc.tile_pool(name=f"kvw{suffix}", bufs=1))
                    gps = pa.enter_context(tc.tile_pool(name=f"gpsk{suffix}", bufs=3, space="PSUM"))
                    kps = pa.enter_context(tc.tile_pool(name=f"kps{suffix}", bufs=2, space="PSUM"))
                    wk_sb = load_wide(wp, wk_d, D, f"wk{suffix}")
                    bd_sb = load_wide(wp, bd_d, HM, f"bd{suffix}")
                    for b in range(B):
                        xb = inp_b[b]
                        kf = work.tile([128, DC * TB], F32R, tag="kf", name=f"kf{suffix}{b}")
                        for mc in range(DC):
                            ps = gps.tile([128, TB], F32, tag="g", name=f"kps_{suffix}")
                            for kc in range(DC):
                                _mm(nc, ps[:], wk_sb[:, kc * D + mc * 128: kc * D + mc * 128 + 128],
                                    xb[:, kc * TB:(kc + 1) * TB], kc == 0, kc == DC - 1)
                            nc.any.tensor_copy(kf[:, mc * TB:(mc + 1) * TB], ps[:])
                        for ts in range(TB // 128):
                            kpt = kpp.tile([128, HM], F32R, tag="kp", name=f"kp{suffix}_{b}_{ts}")
                            psk = kps.tile([128, HM], F32, tag="kpps", name=f"kpps{suffix}")
                            for kc in range(DC):
                                _mm(nc, psk[:],
                                    kf[:, kc * TB + ts * 128: kc * TB + ts * 128 + 128],
                                    bd_sb[:, kc * HM: (kc + 1) * HM],
                                    kc == 0, kc == DC - 1)
                            nc.vector.tensor_scalar(kpt[:], psk[:], 0.0, STAB,
                                                    AluOpType.max, AluOpType.add)
                            kp_t[(b, ts)] = kpt

                with ExitStack() as pb:
                    wp = pb.enter_context(tc.tile_pool(name=f"wvp{suffix}", bufs=1))
                    vp = pb.enter_context(tc.tile_pool(name=f"vtp{suffix}", bufs=3))
                    kvo = pb.enter_context(tc.tile_pool(name=f"kvo{suffix}", bufs=2))
                    gps = pb.enter_context(tc.tile_pool(name=f"gpsv{suffix}", bufs=3, space="PSUM"))
                    kvps = pb.enter_context(tc.tile_pool(name=f"kvps{suffix}", bufs=1, space="PSUM"))
                    wv_sb = load_wide(wp, wv_d, D, f"wv{suffix}")
                    for b in range(B):
                        xb = inp_b[b]
                        kvjA = kvps.tile([128, 512], F32, tag="A", name=f"kvjA{suffix}")
                        kvjB = kvps.tile([128, 512], F32, tag="Bt", name=f"kvjB{suffix}")
                        kvjS = kvps.tile([128, 8], F32, tag="S", name=f"kvjS{suffix}")
                        for ts in range(TB // 128):
                            vt = vp.tile([128, D], F32R, tag="vt", name=f"vt{suffix}")
                            for nb in range(2):
                                ps = gps.tile([128, 512], F32, tag="g", name=f"vps_{suffix}")
                                for kc in range(DC):
                                    _mm(nc, ps[:],
                                        xb[:, kc * TB + ts * 128: kc * TB + ts * 128 + 128],
                                        wv_sb[:, kc * D + nb * 512: kc * D + nb * 512 + 512],
                                        kc == 0, kc == DC - 1)
                                nc.any.tensor_copy(vt[:, nb * 512:(nb + 1) * 512], ps[:])

                            kpt = kp_t[(b, ts)]
                            first, last = ts == 0, ts == TB // 128 - 1
                            _mm(nc, kvjA[:], kpt[:], vt[:, 0:512], first, last, True)
                            _mm(nc, kvjB[:], kpt[:], vt[:, 512:1024], first, last, True)
                            _mm(nc, kvjS[:], kpt[:], ones_col[:], first, last, True)

                        kvj = kvo.tile([128, D + 1], F32, tag="kvj", name=f"kvj{suffix}")
                        nc.any.tensor_copy(kvj[:, 0:512], kvjA[:])
                        nc.any.tensor_copy(kvj[:, 512:1024], kvjB[:])
                        nc.any.tensor_copy(kvj[:, 1024:1025], kvjS[:, 0:1])
                        nc.sync.dma_start(arin[:, b * (D + 1):(b + 1) * (D + 1)], kvj[:])
                        if arout_b is not None:
                            allreduce_b(arin, arout_b, b)

        def q_phase(wq_d, bd_d, inp_b, qp_fm, suffix, from_dram=None):
            """Q projection + qp features -> qp_fm [128, T] feature-major."""
            with ExitStack() as ph:
                wp = ph.enter_context(tc.tile_pool(name=f"wq{suffix}", bufs=1))
                work = ph.enter_context(tc.tile_pool(name=f"qw{suffix}", bufs=2))
                gps = ph.enter_context(tc.tile_pool(name=f"qgps{suffix}", bufs=2, space="PSUM"))
                qps_ = ph.enter_context(tc.tile_pool(name=f"qpps{suffix}", bufs=2, space="PSUM"))

                wq_sb = load_wide(wp, wq_d, D, f"wqw{suffix}")
                bd_sb = load_wide(wp, bd_d, HM, f"bdq{suffix}")

                for b in range(B):
                    if from_dram is not None:
                        xb = work.tile([128, DC * TB], F32R, tag="encb", name=f"encb{suffix}")
                        for kc in range(DC):
                            nc.sync.dma_start(
                                xb[:, kc * TB:(kc + 1) * TB],
                                from_dram[kc * 128:(kc + 1) * 128, b * TB:(b + 1) * TB])
                    else:
                        xb = inp_b[b]
                    qf = work.tile([128, DC * TB], F32R, tag="qf", name=f"qf{suffix}", bufs=1)
                    for mc in range(DC):
                        ps = gps.tile([128, TB], F32, tag="g", name=f"qps_{suffix}")
                        for kc in range(DC):
                            _mm(nc, ps[:], wq_sb[:, kc * D + mc * 128: kc * D + mc * 128 + 128],
                                xb[:, kc * TB:(kc + 1) * TB], kc == 0, kc == DC - 1)
                        nc.any.tensor_copy(qf[:, mc * TB:(mc + 1) * TB], ps[:])
                    pq = qps_.tile([128, TB], F32, tag="qp", name=f"qpps_{suffix}")
                    for kc in range(DC):
                        _mm(nc, pq[:], bd_sb[:, kc * HM:(kc + 1) * HM],
                            qf[:, kc * TB:(kc + 1) * TB], kc == 0, kc == DC - 1)
                    nc.vector.tensor_scalar(qp_fm[:, b * TB:(b + 1) * TB], pq[:],
                                            0.0, STAB, AluOpType.max, AluOpType.add)

        def favor_out_phase(wo_d, arout, qp_fm, inp_b, out_b_list, ln_idx, suffix,
                            spill_to=None, out_dt=F32R):
            """o = (qp/z) @ kv per head, o-proj, residual + LN -> out_b tiles.

            LN tail (abc/bbc broadcast + scale) of batch b is deferred until
            after batch b+1's matmuls are enqueued, so the PE never waits on
            the DVE stats chain."""
            with ExitStack() as ph:
                wp = ph.enter_context(tc.tile_pool(name=f"wo{suffix}", bufs=1))
                kvp = ph.enter_context(tc.tile_pool(name=f"kvi{suffix}", bufs=2))
                bdkvp = ph.enter_context(tc.tile_pool(name=f"bdkv{suffix}", bufs=2))
                fv = ph.enter_context(tc.tile_pool(name=f"fv{suffix}", bufs=2))
                ofm = ph.enter_context(tc.tile_pool(name=f"ofm{suffix}", bufs=1))
                r1p = ph.enter_context(tc.tile_pool(name=f"r1{suffix}", bufs=2))
                sqp = ph.enter_context(tc.tile_pool(name=f"sq{suffix}", bufs=2))
                stp = ph.enter_context(tc.tile_pool(name=f"st{suffix}", bufs=6))
                gps = ph.enter_context(tc.tile_pool(name=f"ogps{suffix}", bufs=3, space="PSUM"))
                sps = ph.enter_context(tc.tile_pool(name=f"osps{suffix}", bufs=5, space="PSUM"))

                wo_sb = load_wide(wp, wo_d, D, f"wow{suffix}", dt=BF16)
                kvmask = kvp.tile([HM, D], F32, tag="kvmask", name=f"kvmask{suffix}")
                nc.sync.dma_start(kvmask[:], kvmask_d[:])

                pend = []

                def finish(b, r1, a_, bb):
                    abc = sps.tile([128, TB], F32, tag="s", name=f"abc{suffix}")
                    _mm(nc, abc[:], ones_row[:], a_[:], True, True)
                    bbc = sps.tile([128, TB], F32, tag="s", name=f"bbc{suffix}")
                    _mm(nc, bbc[:], ones_row[:], bb[:], True, True)
                    ob = resid.tile([128, DC * TB], out_dt, tag="resid",
                                    name=f"out{ln_idx}_{b}")
                    for mc in range(DC):
                        tpm = sqp.tile([128, TB], F32, tag="sq", name=f"tpm{suffix}")
                        nc.vector.tensor_tensor(tpm[:], r1[:, mc * TB:(mc + 1) * TB],
                                                abc[:], AluOpType.mult)
                        nc.vector.tensor_tensor(tpm[:], tpm[:], bbc[:], AluOpType.add)
                        nc.scalar.activation(ob[:, mc * TB:(mc + 1) * TB], tpm[:],
                                             AF.Identity, bias=beslice(ln_idx)[:, mc:mc + 1],
                                             scale=gslice(ln_idx)[:, mc:mc + 1])
                    if spill_to is not None:
                        for kc in range(DC):
                            nc.sync.dma_start(
                                spill_to[kc * 128:(kc + 1) * 128, b * TB:(b + 1) * TB],
                                ob[:, kc * TB:(kc + 1) * TB])
                    out_b_list.append(ob)

                def prepA(b):
                    """kvb load + bdkv + qpk + zps matmul (PE leaves quickly)."""
                    kvb = kvp.tile([HM, D + 1], F32, tag="kvb", name=f"kvb{suffix}")
                    nc.sync.dma_start(kvb[:], arout[b][:])
                    bdkv = bdkvp.tile([HM, D], F32R, tag="bdkv", name=f"bdkv_{suffix}")
                    nc.vector.tensor_tensor(bdkv[:], kvb[:, 0:D], kvmask[:],
                                            AluOpType.mult)
                    qpk = fv.tile([128, TB], F32R, tag="qpk", name=f"qpk{suffix}")
                    nc.vector.tensor_scalar(qpk[:], qp_fm[:, b * TB:(b + 1) * TB],
                                            kvb[:, D:D + 1], None,
                                            AluOpType.mult)
                    zps = sps.tile([H, TB], F32, tag="s", name=f"z{suffix}")
                    _mm(nc, zps[:], e16T[:], qpk[:], True, True)
                    return bdkv, zps

                def prepB(b, zps):
                    """reciprocal chain + zbc broadcast + qps_t."""
                    rz = fv.tile([H, TB], F32R, tag="rz", name=f"rz{suffix}")
                    nc.vector.reciprocal(rz[:], zps[:])
                    if NEWTON:
                        t1 = fv.tile([H, TB], F32, tag="nt1", name=f"nt1{suffix}")
                        nc.vector.tensor_tensor(t1[:], zps[:], rz[:], AluOpType.mult)
                        nc.vector.tensor_scalar(t1[:], t1[:], -1.0, 2.0,
                                                AluOpType.mult, AluOpType.add)
                        nc.vector.tensor_tensor(rz[:], rz[:], t1[:], AluOpType.mult)
                    zbc = sps.tile([128, TB], F32, tag="s", name=f"zbc{suffix}")
                    _mm(nc, zbc[:], e16[:], rz[:], True, True)
                    qps_t = fv.tile([128, TB], F32R, tag="qps", name=f"qps{suffix}")
                    nc.vector.tensor_tensor(qps_t[:], qp_fm[:, b * TB:(b + 1) * TB],
                                            zbc[:], AluOpType.mult)
                    return qps_t

                _a = prepA(0)
                prepped = (_a[0], prepB(0, _a[1]))
                for b in range(B):
                    bdkv, qps_t = prepped

                    # o feature-major via block-diag kv
                    of = ofm.tile([128, DC * TB], BF16, tag="of", name=f"of{suffix}")
                    for c in range(DC):
                        ps = gps.tile([128, TB], F32, tag="g", name=f"ops_{suffix}")
                        _mm(nc, ps[:], bdkv[:, c * 128:(c + 1) * 128], qps_t[:],
                            True, True)
                        nc.any.tensor_copy(of[:, c * TB:(c + 1) * TB], ps[:])

                    # o-proj + residual + LN stats
                    r1 = r1p.tile([128, DC * TB], F32R, tag="r1", name=f"r1{suffix}")
                    Sp = sps.tile([1, TB], F32, tag="s", name=f"S{suffix}")
                    SSp = sps.tile([1, TB], F32, tag="s", name=f"SS{suffix}")
                    for mc in range(DC):
                        ps = gps.tile([128, TB], F32, tag="g", name=f"ojps_{suffix}")
                        for kc in range(DC):
                            nc.tensor.matmul(
                                ps[:], wo_sb[:, kc * D + mc * 128: kc * D + mc * 128 + 128],
                                of[:, kc * TB:(kc + 1) * TB],
                                start=(kc == 0), stop=(kc == DC - 1))
                        nc.vector.tensor_tensor(r1[:, mc * TB:(mc + 1) * TB], ps[:],
                                                inp_b[b][:, mc * TB:(mc + 1) * TB],
                                                AluOpType.add)
                        sq = sqp.tile([128, TB], F32R, tag="sq", name=f"sq{suffix}")
                        nc.scalar.activation(sq[:], r1[:, mc * TB:(mc + 1) * TB], AF.Square)
                        _mm(nc, Sp[:], ones_col[:, 0:1], r1[:, mc * TB:(mc + 1) * TB],
                            mc == 0, mc == DC - 1, True)
                        _mm(nc, SSp[:], ones_col[:, 0:1], sq[:], mc == 0, mc == DC - 1, True)

                    # stats -> a (rstd), bb (-m*rstd)
                    mneg = stp.tile([1, TB], F32, tag="st", name=f"mneg{suffix}")
                    nc.vector.tensor_scalar(mneg[:], Sp[:], -1.0 / D, None, AluOpType.mult)
                    m2 = stp.tile([1, TB], F32, tag="st", name=f"m2{suffix}")
                    nc.vector.tensor_tensor(m2[:], mneg[:], mneg[:], AluOpType.mult)
                    ve = stp.tile([1, TB], F32, tag="st", name=f"ve{suffix}")
                    nc.vector.scalar_tensor_tensor(ve[:], in0=SSp[:], scalar=1.0 / D,
                                                   in1=m2[:], op0=AluOpType.mult,
                                                   op1=AluOpType.subtract)
                    sqv = stp.tile([1, TB], F32, tag="st", name=f"sqv{suffix}")
                    nc.scalar.activation(sqv[:], ve[:], AF.Sqrt, bias=eps_t[:])
                    a_ = stp.tile([1, TB], F32R, tag="st", name=f"a{suffix}")
                    nc.vector.reciprocal(a_[:], sqv[:])
                    if NEWTON:
                        n1 = stp.tile([1, TB], F32, tag="st", name=f"n1{suffix}")
                        nc.vector.tensor_tensor(n1[:], a_[:], a_[:], AluOpType.mult)
                        n2 = stp.tile([1, TB], F32, tag="st", name=f"n2{suffix}")
                        nc.vector.scalar_tensor_tensor(n2[:], in0=ve[:], scalar=EPS_LN,
                                                       in1=n1[:], op0=AluOpType.add,
                                                       op1=AluOpType.mult)
                        nc.vector.tensor_scalar(n2[:], n2[:], -0.5, 1.5,
                                                AluOpType.mult, AluOpType.add)
                        nc.vector.tensor_tensor(a_[:], a_[:], n2[:], AluOpType.mult)
                    bb = stp.tile([1, TB], F32R, tag="st", name=f"bb{suffix}")
                    nc.vector.tensor_tensor(bb[:], mneg[:], a_[:], AluOpType.mult)

                    if pend:
                        finish(*pend.pop())
                    pend.append((b, r1, a_, bb))
                    if b + 1 < B:
                        _a = prepA(b + 1)
                        prepped = (_a[0], prepB(b + 1, _a[1]))
                finish(*pend.pop())

        def allreduce_b(arin_b, arout_b, b):
            if VARIANT == 'noar':
                nc.sync.dma_start(arout_b[b][:], arin_b[b][:])
                return
            nc.gpsimd.collective_compute(
                "AllReduce", AluOpType.add,
                replica_groups=[list(range(NCORES))],
                ins=[arin_b[b][:]], outs=[arout_b[b][:]])

        if VARIANT == 'ffn':
            # feed x directly into the FFN path (cast to bf16 for the spill)
            out2_b = x_b
            with ExitStack() as phx:
                xcp = phx.enter_context(tc.tile_pool(name="xcast", bufs=3))
                for b in range(B):
                    for kc in range(DC):
                        xh = xcp.tile([128, TB], BF16, tag="xh", name="xh")
                        nc.vector.tensor_copy(xh[:], x_b[b][:, kc * TB:(kc + 1) * TB])
                        nc.sync.dma_start(
                            out2_spill[kc * 128:(kc + 1) * 128, b * TB:(b + 1) * TB],
                            xh[:])
        else:
            # =================== attention 1 (self) ===================
            kv_phase(wk1, wv1, bd1, x_b, arin1_b, "a1", arout_b=arout1_b)
            qp1 = qp_pool.tile([HM, T], F32R, tag="qp", name="qp1")
            q_phase(wq1, bd1, x_b, qp1, "a1")
            out1_b = []
            favor_out_phase(wo1, arout1_b, qp1, x_b, out1_b, 0, "a1")

            if VARIANT == 'attn1':
                out2_b = out1_b
            else:
                # =============== attention 2 (cross: q enc, kv out1) =========
                kv_phase(wk2, wv2, bd2, out1_b, arin2_b, "a2", arout_b=arout2_b)
                qp2 = qp_pool.tile([HM, T], F32R, tag="qp", name="qp2")
                q_phase(wq2, bd2, None, qp2, "a2", from_dram=encT)
                out2_b = []
                favor_out_phase(wo2, arout2_b, qp2, out1_b, out2_b, 1, "a2",
                                spill_to=out2_spill, out_dt=BF16)

        if VARIANT in ('attn1', 'attns'):
            # write out2_b straight to outT and stop
            with ExitStack() as ph:
                for b in range(B):
                    for kc in range(DC):
                        nc.sync.dma_start(
                            outT[kc * 128:(kc + 1) * 128, b * TB:(b + 1) * TB],
                            out2_b[b][:, kc * TB:(kc + 1) * TB].bitcast(F32))
            mid.close()
            nc.compile()
            return nc

        # =================== FFN (fused, no h spill) ===================
        # out2 lives in DRAM as bf16 [D, T]. Per 512-token block: h computed
        # in SBUF bf16, consumed immediately by W2. Both weights resident bf16.
        mid.close()
        TB3 = 512
        KD = DFF // 128  # 32
        with ExitStack() as ph:
            wp = ph.enter_context(tc.tile_pool(name="wffn", bufs=1))
            o2p = ph.enter_context(tc.tile_pool(name="o2p", bufs=2))
            hsbp = ph.enter_context(tc.tile_pool(name="hsb", bufs=1))
            ep = ph.enter_context(tc.tile_pool(name="ep", bufs=3))
            r3p = ph.enter_context(tc.tile_pool(name="r3p", bufs=1))
            sqp = ph.enter_context(tc.tile_pool(name="sq3", bufs=2))
            stp = ph.enter_context(tc.tile_pool(name="st3", bufs=8))
            o3p = ph.enter_context(tc.tile_pool(name="o3p", bufs=2))
            bp = ph.enter_context(tc.tile_pool(name="bp", bufs=1))
            fps_h = ph.enter_context(tc.tile_pool(name="fpsh", bufs=2, space="PSUM"))
            fps_st = ph.enter_context(tc.tile_pool(name="fpsst", bufs=2, space="PSUM"))
            fps_rt = ph.enter_context(tc.tile_pool(name="fpsrt", bufs=4, space="PSUM"))

            w1p = ph.enter_context(tc.tile_pool(name="w1p", bufs=2))
            w2_sb = wp.tile([128, KD * D], BF16, name="w2sb")
            b1row = bp.tile([1, DFF], BF16, name="b1row")
            nc.sync.dma_start(b1row[:], b1r_d[:])
            b2row = bp.tile([1, D], BF16, name="b2row")
            nc.sync.dma_start(b2row[:], b2r_d[:])
            ones_h = bp.tile([1, TB3], BF16, name="ones_h")
            nc.vector.memset(ones_h[:], 1.0)

            for t3 in range(T // TB3):
                o2c = o2p.tile([128, DC * TB3], BF16, tag="o2c", name="o2c")
                for kc in range(DC):
                    nc.sync.dma_start(o2c[:, kc * TB3:(kc + 1) * TB3],
                                      out2_spill[kc * 128:(kc + 1) * 128,
                                                 t3 * TB3:(t3 + 1) * TB3])
                h_sb = hsbp.tile([128, KD * TB3], BF16, tag="h", name="hsb_t")
                for kdg in range(DFF // 512):
                    w1c = w1p.tile([128, DC * 512], BF16, tag="w1c", name="w1c")
                    for kc in range(DC):
                        nc.sync.dma_start(w1c[:, kc * 512:(kc + 1) * 512],
                                          w1[kc * 128:(kc + 1) * 128,
                                             kdg * 512:(kdg + 1) * 512])
                    for j in range(4):
                        kd = kdg * 4 + j
                        ps = fps_h.tile([128, TB3], F32, tag="h", name="hps_t")
                        for kc in range(DC):
                            nc.tensor.matmul(
                                ps[:],
                                w1c[:, kc * 512 + j * 128: kc * 512 + j * 128 + 128],
                                o2c[:, kc * TB3:(kc + 1) * TB3],
                                start=(kc == 0), stop=False)
                        nc.tensor.matmul(ps[:], b1row[0:1, kd * 128:(kd + 1) * 128],
                                         ones_h[:], start=False, stop=True)
                        # ELU: h = min(exp(u) - 1, max(u, 0))
                        e_ = ep.tile([128, TB3], F32, tag="e", name="e_t")
                        nc.scalar.activation(e_[:], ps[:], AF.Exp)
                        t_ = ep.tile([128, TB3], F32, tag="t", name="t_t")
                        nc.vector.tensor_scalar(t_[:], ps[:], 0.0, None, AluOpType.max)
                        nc.vector.scalar_tensor_tensor(
                            h_sb[:, kd * TB3:(kd + 1) * TB3], in0=e_[:], scalar=1.0,
                            in1=t_[:], op0=AluOpType.subtract, op1=AluOpType.min)

                if t3 == 0:
                    # W2 streams in while FFN1(t3=0) computes; per-chunk deps
                    for kc in range(KD):
                        nc.sync.dma_start(w2_sb[:, kc * D:(kc + 1) * D],
                                          w2[kc * 128:(kc + 1) * 128, :])
                # W2 in two half-groups of 4 d-chunks (PSUM: 4 rt banks)
                r3 = r3p.tile([128, DC * TB3], F32R, tag="r3s", name="r3s")
                Sp = fps_st.tile([1, TB3], F32, tag="st", name="S3")
                SSp = fps_st.tile([1, TB3], F32, tag="st", name="SS3")
                for half in range(2):
                    rt = [fps_rt.tile([128, TB3], F32, tag="rt", name=f"rt{i}")
                          for i in range(4)]
                    for kc in range(KD):
                        for i in range(4):
                            c = half * 4 + i
                            nc.tensor.matmul(
                                rt[i][:],
                                w2_sb[:, kc * D + c * 128: kc * D + c * 128 + 128],
                                h_sb[:, kc * TB3:(kc + 1) * TB3],
                                start=(kc == 0), stop=False, skip_group_check=True)
                    for i in range(4):
                        c = half * 4 + i
                        nc.tensor.matmul(rt[i][:], b2row[0:1, c * 128:(c + 1) * 128],
                                         ones_h[:], start=False, stop=True,
                                         skip_group_check=True)
                    for i in range(4):
                        c = half * 4 + i
                        nc.vector.tensor_tensor(r3[:, c * TB3:(c + 1) * TB3], rt[i][:],
                                                o2c[:, c * TB3:(c + 1) * TB3],
                                                AluOpType.add)
                        sq = sqp.tile([128, TB3], F32R, tag="sq3", name="sq3t")
                        nc.scalar.activation(sq[:], r3[:, c * TB3:(c + 1) * TB3],
                                             AF.Square)
                        _mm(nc, Sp[:], ones_col[:, 0:1], r3[:, c * TB3:(c + 1) * TB3],
                            c == 0, c == DC - 1, True)
                        _mm(nc, SSp[:], ones_col[:, 0:1], sq[:], c == 0, c == DC - 1,
                            True)

                mneg = stp.tile([1, TB3], F32, tag="st3", name="mneg3")
                nc.vector.tensor_scalar(mneg[:], Sp[:], -1.0 / D, None, AluOpType.mult)
                m2 = stp.tile([1, TB3], F32, tag="st3", name="m23")
                nc.vector.tensor_tensor(m2[:], mneg[:], mneg[:], AluOpType.mult)
                ve = stp.tile([1, TB3], F32, tag="st3", name="ve3")
                nc.vector.scalar_tensor_tensor(ve[:], in0=SSp[:], scalar=1.0 / D,
                                               in1=m2[:], op0=AluOpType.mult,
                                               op1=AluOpType.subtract)
                sqv = stp.tile([1, TB3], F32, tag="st3", name="sqv3")
                nc.scalar.activation(sqv[:], ve[:], AF.Sqrt, bias=eps_t[:])
                a_ = stp.tile([1, TB3], F32R, tag="st3", name="a3")
                nc.vector.reciprocal(a_[:], sqv[:])
                if NEWTON:
                    n1 = stp.tile([1, TB3], F32, tag="st3", name="n13")
                    nc.vector.tensor_tensor(n1[:], a_[:], a_[:], AluOpType.mult)
                    n2 = stp.tile([1, TB3], F32, tag="st3", name="n23")
                    nc.vector.scalar_tensor_tensor(n2[:], in0=ve[:], scalar=EPS_LN,
                                                   in1=n1[:], op0=AluOpType.add,
                                                   op1=AluOpType.mult)
                    nc.vector.tensor_scalar(n2[:], n2[:], -0.5, 1.5,
                                            AluOpType.mult, AluOpType.add)
                    nc.vector.tensor_tensor(a_[:], a_[:], n2[:], AluOpType.mult)
                bb = stp.tile([1, TB3], F32R, tag="st3", name="bb3")
                nc.vector.tensor_tensor(bb[:], mneg[:], a_[:], AluOpType.mult)
                abc = fps_st.tile([128, TB3], F32, tag="st", name="abc3")
                _mm(nc, abc[:], ones_row[:], a_[:], True, True)
                bbc = fps_st.tile([128, TB3], F32, tag="st", name="bbc3")
                _mm(nc, bbc[:], ones_row[:], bb[:], True, True)

                for c in range(DC):
                    tpm = sqp.tile([128, TB3], F32, tag="sq3", name="tpm3")
                    nc.vector.tensor_tensor(tpm[:], r3[:, c * TB3:(c + 1) * TB3],
                                            abc[:], AluOpType.mult)
                    nc.vector.tensor_tensor(tpm[:], tpm[:], bbc[:], AluOpType.add)
                    o3 = o3p.tile([128, TB3], F32, tag="o3", name="o3t")
                    nc.scalar.activation(o3[:], tpm[:], AF.Identity,
                                         bias=beslice(2)[:, c:c + 1],
                                         scale=gslice(2)[:, c:c + 1])
                    nc.sync.dma_start(outT[c * 128:(c + 1) * 128,
                                           t3 * TB3:(t3 + 1) * TB3], o3[:])

    nc.compile()
    return nc


def _host_prep(inputs):
    """Build per-core in_maps from full inputs."""
    f32 = np.float32
    x = np.asarray(inputs['x'], f32)
    enc = np.asarray(inputs['enc_output'], f32)

    def bdiag(P):
        bd = np.zeros((D, HM), f32)
        pt = (np.asarray(P, f32) / np.sqrt(M)).T  # [DH, M]
        for h in range(H):
            bd[h * DH:(h + 1) * DH, h * M:(h + 1) * M] = pt
        return bd

    e16T = np.zeros((HM, H), f32)
    e16 = np.zeros((H, HM), f32)
    kvmask = np.zeros((HM, D), f32)
    for h in range(H):
        e16T[h * M:(h + 1) * M, h] = 1.0
        e16[h, h * M:(h + 1) * M] = 1.0
        kvmask[h * M:(h + 1) * M, h * DH:(h + 1) * DH] = 1.0

    gbe = np.zeros((128, 6 * DC), f32)
    for i, nm in enumerate(['g1', 'be1', 'g2', 'be2', 'g3', 'be3']):
        gbe[:, i * DC:(i + 1) * DC] = np.asarray(inputs[nm], f32).reshape(DC, 128).T

    shared = {
        'wq1': np.ascontiguousarray(np.asarray(inputs['Wq1'], f32).reshape(D, D)),
        'wk1': np.ascontiguousarray(np.asarray(inputs['Wk1'], f32).reshape(D, D)),
        'wv1': np.ascontiguousarray(np.asarray(inputs['Wv1'], f32).reshape(D, D)),
        'wo1': np.ascontiguousarray(np.asarray(inputs['Wo1'], f32).reshape(D, D)).astype(ml_dtypes.bfloat16),
        'bd1': bdiag(inputs['P1']),
        'wq2': np.ascontiguousarray(np.asarray(inputs['Wq2'], f32).reshape(D, D)),
        'wk2': np.ascontiguousarray(np.asarray(inputs['Wk2'], f32).reshape(D, D)),
        'wv2': np.ascontiguousarray(np.asarray(inputs['Wv2'], f32).reshape(D, D)),
        'wo2': np.ascontiguousarray(np.asarray(inputs['Wo2'], f32).reshape(D, D)).astype(ml_dtypes.bfloat16),
        'bd2': bdiag(inputs['P2']),
        'e16T': e16T, 'e16': e16, 'kvmask': kvmask,
        'w1': np.ascontiguousarray(np.asarray(inputs['W1'], f32)).astype(ml_dtypes.bfloat16),
        'w2': np.ascontiguousarray(np.asarray(inputs['W2'], f32)).astype(ml_dtypes.bfloat16),
        'b1r': np.asarray(inputs['b1'], f32).reshape(1, DFF).astype(ml_dtypes.bfloat16),
        'b2r': np.asarray(inputs['b2'], f32).reshape(1, D).astype(ml_dtypes.bfloat16),
        'gbe': gbe,
        'ones_col': np.ones((128, 8), f32),
        'ones_row': np.ones((1, 128), f32),
        'ones_tb': np.ones((1, TB), f32),
    }

    in_maps = []
    for i in range(NCORES):
        sl = slice(i * LSH, (i + 1) * LSH)
        m = dict(shared)
        m['xT'] = np.ascontiguousarray(
            x[:, sl, :].transpose(2, 0, 1).reshape(D, T))
        m['encT'] = np.ascontiguousarray(
            enc[:, sl, :].transpose(2, 0, 1).reshape(D, T))
        in_maps.append(m)
    return in_maps


def kernel(**inputs) -> np.ndarray:
    if 'nc' not in _cache:
        _cache['nc'] = build_program()
    nc = _cache['nc']
    in_maps = _host_prep(inputs)
    res = run_bass_kernel_spmd(nc, in_maps, core_ids=list(range(NCORES)))
    out = np.empty((B, L, D), np.float32)
    for i in range(NCORES):
        o = res.results[i]['outT']  # [D, T] feature-major
        out[:, i * LSH:(i + 1) * LSH, :] = o.reshape(D, B, LSH).transpose(1, 2, 0)
    return out


if __name__ == '__main__':
    np.random.seed(0)
    print("building program...")
    build_program()
    print("OK")

